# revision 2
# baseline (speedup 1.0000x reference)
"""AxialAttention kernel for 8 trn2 NeuronCores — full attention on device.

Data-parallel over batch N=16 -> 2 batches per core. The axon tunnel
(~35MB/s) dominates the measured time, so all compute (projections,
relative-embedding logits, softmax over W, sv/sve contractions) runs on
device; x goes up fp32 (softmax of +-870 logits needs fp32 on the q/k
path) and the final output comes back bf16 (6.4MB/core instead of the
baseline's 25.7MB/core fp32 projection). BN affines are folded into
weights/tables on the host:
  qs = BNq(x@Wq)*s_qr[g]        (folded into Wq', shift_q')
  k' = BNk(x@Wk)*s_qk[g]/s_qr[g]
  v' = BNv(x@Wv)*s_sv[ch]
  logit = qs.k' (qk) + qs.qe (qr) + (k'.ke)*sckr[g] (kr), sckr=s_kr*s_qr/s_qk
  sim = softmax_w(logit);  out = sim@v' + sim@(ve*s_sve) + (shift_sv+shift_sve)
The per-g BN shifts of qk/qr/kr cancel in softmax over w (constant along
w). The final +C[ch] shift is added on the host after the gather. The
q/k/logit/softmax path is fp32; sim/v/out are bf16 (error ~0.4%, well
inside the 2e-2 gate).
"""

import numpy as np

K = 56
G = 8
CIN = 512
COUT = 512
EPS = 1e-3
NCORES = 8
NB = 16
NLOC = NB // NCORES          # 2 batches per core
PIXB = K * K                 # 3136 pixels per batch
NT = 7                       # 3136 = 7 * 448
FT = 448
IQ = 14                      # i-quarter size
_state = {}


def _build():
    import concourse.bacc as bacc
    import concourse.mybir as mybir
    import concourse.tile as tile

    bf = mybir.dt.bfloat16
    f32 = mybir.dt.float32
    EXP = mybir.ActivationFunctionType.Exp
    ADD = mybir.AluOpType.add
    MAX = mybir.AluOpType.max
    AX = mybir.AxisListType.X

    nc = bacc.Bacc("TRN2", target_bir_lowering=False, debug=False)
    xt = nc.declare_dram_parameter("xt", [128, 4, NLOC * PIXB], f32,
                                   isOutput=False)
    wt = nc.declare_dram_parameter("wt", [128, 4, 1024], f32, isOutput=False)
    qed = nc.declare_dram_parameter("qed", [32, PIXB], f32, isOutput=False)
    ked = nc.declare_dram_parameter("ked", [32, PIXB], f32, isOutput=False)
    vrd = nc.declare_dram_parameter("vrd", [128, 512], bf, isOutput=False)
    cvd = nc.declare_dram_parameter("cvd", [128, 8], f32, isOutput=False)
    scd = nc.declare_dram_parameter("scd", [IQ, 448], f32, isOutput=False)
    idd = nc.declare_dram_parameter("idd", [128, 128], bf, isOutput=False)
    yt = nc.declare_dram_parameter("yt", [NLOC, K, K * 512], bf, isOutput=True)

    with tile.TileContext(nc) as tc:
        with tc.tile_pool(name="const", bufs=1) as cp, \
             tc.tile_pool(name="dram", bufs=1, space="DRAM") as dp:
            vrt = cp.tile([128, 512], bf)          # [r, g*64+c] scaled v_rel
            nc.sync.dma_start(out=vrt[:], in_=vrd[:])
            cvec = cp.tile([128, 8], f32)          # shifts per ch-tile
            nc.sync.dma_start(out=cvec[:], in_=cvd[:])
            sckr = cp.tile([IQ, 448], f32)
            nc.sync.dma_start(out=sckr[:], in_=scd[:])

            for b in range(NLOC):
                qtd = dp.tile([32, G, PIXB], f32, tag="qtd")   # [c,g,(i w)]
                ktd = dp.tile([32, G, PIXB], f32, tag="ktd")
                dv2p = dp.tile([PIXB, 512], bf, tag="dv2p")    # [(j w), ch]
                ds0 = dp.tile([K, IQ * 448], bf, tag="ds0")
                ds1 = dp.tile([K, IQ * 448], bf, tag="ds1")
                ds2 = dp.tile([K, IQ * 448], bf, tag="ds2")
                ds3 = dp.tile([K, IQ * 448], bf, tag="ds3")
                dsim = [ds0, ds1, ds2, ds3]

                # ---- phase A: projection, q/k/v to DRAM in new layouts ---
                with tc.tile_pool(name="pa", bufs=1) as pa, \
                     tc.tile_pool(name="xs", bufs=2) as xsp, \
                     tc.tile_pool(name="tmpp", bufs=3) as tp, \
                     tc.tile_pool(name="psA", bufs=4, space="PSUM") as psA, \
                     tc.tile_pool(name="psT", bufs=4, space="PSUM") as psT:
                    w_sb = pa.tile([128, 4, 1024], f32, tag="w")
                    nc.sync.dma_start(out=w_sb[:], in_=wt[:])
                    idn = pa.tile([128, 128], bf, tag="idn")
                    nc.sync.dma_start(out=idn[:], in_=idd[:])
                    for n in range(NT):
                        sl = slice(n * FT, (n + 1) * FT)
                        x_sb = xsp.tile([128, 4, FT], f32, tag="x")
                        nc.sync.dma_start(
                            out=x_sb[:],
                            in_=xt[:, :, b * PIXB + n * FT:
                                   b * PIXB + (n + 1) * FT])
                        for m in range(8):
                            ps = psA.tile([128, FT], f32, tag="ps")
                            for kk in range(4):
                                nc.tensor.matmul(
                                    ps[:],
                                    lhsT=w_sb[:, kk, m * 128:(m + 1) * 128],
                                    rhs=x_sb[:, kk, :],
                                    start=(kk == 0), stop=(kk == 3))
                            if m < 4:  # q (m 0,1) / k (m 2,3): fp32 -> DRAM
                                tmq = tp.tile([128, FT], f32, tag="tmq")
                                nc.vector.tensor_scalar_add(
                                    out=tmq[:], in0=ps[:],
                                    scalar1=cvec[:, m:m + 1])
                                dst = qtd if m < 2 else ktd
                                for gg in range(4):
                                    g = (m % 2) * 4 + gg
                                    nc.sync.dma_start(
                                        out=dst[:, g, sl],
                                        in_=tmq[gg * 32:(gg + 1) * 32, :])
                            else:      # v: bf16, transpose -> pix-major DRAM
                                tmv = tp.tile([128, FT], bf, tag="tmv")
                                nc.vector.tensor_scalar_add(
                                    out=tmv[:], in0=ps[:],
                                    scalar1=cvec[:, m:m + 1])
                                for cc in range(4):
                                    fs = slice(cc * 112, (cc + 1) * 112)
                                    pst = psT.tile([112, 128], bf, tag="pst")
                                    nc.tensor.transpose(pst[:], tmv[:, fs],
                                                        idn[:])
                                    tt2 = tp.tile([112, 128], bf, tag="tt2")
                                    nc.vector.tensor_copy(out=tt2[:],
                                                          in_=pst[:])
                                    nc.sync.dma_start(
                                        out=dv2p[n * FT + cc * 112:
                                                 n * FT + (cc + 1) * 112,
                                                 (m - 4) * 128:(m - 3) * 128],
                                        in_=tt2[:])

                # ---- phase B: logits + softmax per i-quarter -------------
                with tc.tile_pool(name="pbk", bufs=1) as pbk, \
                     tc.tile_pool(name="pbq", bufs=1) as pbq, \
                     tc.tile_pool(name="tmpb", bufs=3) as tb, \
                     tc.tile_pool(name="psqk", bufs=4, space="PSUM") as psqk, \
                     tc.tile_pool(name="psqr", bufs=2, space="PSUM") as psqr, \
                     tc.tile_pool(name="pskr", bufs=2, space="PSUM") as pskr:
                    kt_sb = pbk.tile([32, G, K, K], f32, tag="kt")  # [c,g,j,w]
                    nc.sync.dma_start(
                        out=kt_sb[:],
                        in_=ktd.rearrange("c g (j w) -> c g j w", w=K))
                    for iq in range(4):
                        i0 = iq * IQ
                        psl = slice(i0 * K, (i0 + IQ) * K)
                        qt_q = pbq.tile([32, G, IQ, K], f32, tag="qtq")
                        nc.sync.dma_start(
                            out=qt_q[:],
                            in_=qtd[:, :, psl].rearrange(
                                "c g (i w) -> c g i w", w=K))
                        qe_q = pbq.tile([32, IQ, K], f32, tag="qeq")  # [c,i,j]
                        nc.sync.dma_start(
                            out=qe_q[:],
                            in_=qed[:, psl].rearrange("c (i j) -> c i j", j=K))
                        ke_q = pbq.tile([32, K, IQ], f32, tag="keq")  # [c,j,i]
                        nc.sync.dma_start(
                            out=ke_q[:],
                            in_=ked.rearrange("c (j i) -> c j i",
                                              i=K)[:, :, i0:i0 + IQ])
                        L = pbq.tile([K, IQ, G, K], f32, tag="L")
                        # kr first: initializes all of L via DMA scatter
                        for j in range(K):
                            ps = pskr.tile([IQ, 448], f32, tag="kr")
                            nc.tensor.matmul(
                                ps[:], lhsT=ke_q[:, j, :],
                                rhs=kt_sb[:, :, j, :], start=True, stop=True)
                            krt = tb.tile([IQ, 448], f32, tag="krt")
                            nc.vector.tensor_mul(out=krt[:], in0=ps[:],
                                                 in1=sckr[:])
                            nc.sync.dma_start(out=L[j:j + 1], in_=krt[:])
                        # qr: contiguous adds
                        for r in range(IQ):
                            ps = psqr.tile([K, 448], f32, tag="qr")
                            nc.tensor.matmul(
                                ps[:], lhsT=qe_q[:, r, :],
                                rhs=qt_q[:, :, r, :], start=True, stop=True)
                            Lv = L[:, r].rearrange("p g w -> p (g w)")
                            nc.vector.tensor_add(out=Lv, in0=Lv, in1=ps[:])
                        # qk: strided adds
                        for g in range(G):
                            for w in range(K):
                                ps = psqk.tile([K, IQ], f32, tag="qk")
                                nc.tensor.matmul(
                                    ps[:], lhsT=kt_sb[:, g, :, w],
                                    rhs=qt_q[:, g, :, w],
                                    start=True, stop=True)
                                nc.vector.tensor_add(
                                    out=L[:, :, g, w], in0=L[:, :, g, w],
                                    in1=ps[:])
                        # softmax over w (innermost dim)
                        red = tb.tile([K, IQ, G], f32, tag="red")
                        nc.vector.tensor_reduce(out=red[:], in_=L[:],
                                                axis=AX, op=MAX)
                        nc.vector.tensor_sub(
                            out=L[:], in0=L[:],
                            in1=red[:, :, :, None].broadcast_to(
                                [K, IQ, G, K]))
                        nc.scalar.activation(out=L[:], in_=L[:], func=EXP)
                        nc.vector.tensor_reduce(out=red[:], in_=L[:],
                                                axis=AX, op=ADD)
                        nc.vector.reciprocal(out=red[:], in_=red[:])
                        sim = pbq.tile([K, IQ, G, K], bf, tag="sim")
                        nc.vector.tensor_mul(
                            out=sim[:], in0=L[:],
                            in1=red[:, :, :, None].broadcast_to(
                                [K, IQ, G, K]))
                        nc.sync.dma_start(out=dsim[iq][:], in_=sim[:])

                # ---- phase C: sv + sve + output --------------------------
                with tc.tile_pool(name="pc", bufs=1) as pc, \
                     tc.tile_pool(name="tmpc", bufs=3) as tcp_, \
                     tc.tile_pool(name="psv", bufs=4, space="PSUM") as psv, \
                     tc.tile_pool(name="pse", bufs=4, space="PSUM") as pse:
                    vt_sb = pc.tile([K, K, 512], bf, tag="vt2")
                    nc.sync.dma_start(
                        out=vt_sb[:],
                        in_=dv2p.rearrange("(j w) c -> j w c", j=K))
                    simc = pc.tile([K, 4, IQ, G, K], bf, tag="simc")
                    for iq in range(4):
                        nc.sync.dma_start(out=simc[:, iq], in_=dsim[iq][:])
                    out_sb = pc.tile([64, K, 512], bf, tag="out")
                    # sve per i: 8 matmuls -> [w, (g c)] -> row pad(i) of OUT
                    for i in range(K):
                        iq, r = divmod(i, IQ)
                        vew = tcp_.tile([K, 512], bf, tag="vew")
                        nc.sync.dma_start(out=vew[:],
                                          in_=vrt[55 - i:111 - i, :])
                        ps = pse.tile([K, 512], f32, tag="sve")
                        for g in range(G):
                            nc.tensor.matmul(
                                ps[:, g * 64:(g + 1) * 64],
                                lhsT=simc[:, iq, r, g, :],
                                rhs=vew[:, g * 64:(g + 1) * 64],
                                start=True, stop=True)
                        svet = tcp_.tile([K, 512], bf, tag="svet")
                        nc.vector.tensor_copy(out=svet[:], in_=ps[:])
                        rr = (i % 28) + 32 * (i // 28)
                        nc.sync.dma_start(out=out_sb[rr:rr + 1, :, :],
                                          in_=svet[:])
                    # sv per (w, half): quarters paired -> psum rows 0/32
                    for w in range(K):
                        ps = psv.tile([64, 512], f32, tag="sv")
                        for hh in range(2):
                            for g in range(G):
                                nc.tensor.matmul(
                                    ps[32 * hh:32 * hh + 28,
                                       g * 64:(g + 1) * 64],
                                    lhsT=simc[:, 2 * hh:2 * hh + 2, :, g, w
                                              ].rearrange("p a b -> p (a b)"),
                                    rhs=vt_sb[:, w, g * 64:(g + 1) * 64],
                                    start=True, stop=True)
                        nc.vector.tensor_add(out=out_sb[:, w, :],
                                             in0=out_sb[:, w, :], in1=ps[:])
                    nc.sync.dma_start(out=yt[b, 0:28], in_=out_sb[0:28])
                    nc.sync.dma_start(out=yt[b, 28:K], in_=out_sb[32:60])
    nc.compile()
    return nc


def _prep_host(Wq, Wk, Wv, q_rel, k_rel, v_rel,
               p_q, p_k, p_v, p_qk, p_qr, p_kr, p_sv, p_sve):
    import ml_dtypes
    bf16 = ml_dtypes.bfloat16

    def aff(p):
        g, b, m, v = np.asarray(p, np.float32)
        s = g / np.sqrt(v + EPS)
        return s, b - m * s

    sq, shq = aff(p_q); sk, shk = aff(p_k); sv_, shv = aff(p_v)
    sqk, _ = aff(p_qk); sqr, _ = aff(p_qr); skr, _ = aff(p_kr)
    ssv, shsv = aff(p_sv); ssve, shsve = aff(p_sve)
    g256 = np.repeat(np.arange(G), 32)
    g512 = np.repeat(np.arange(G), 64)

    fq = sq * sqr[g256]
    fk = sk * sqk[g256] / sqr[g256]
    fv = sv_ * ssv
    wcat = np.concatenate([np.asarray(Wq, np.float32) * fq,
                           np.asarray(Wk, np.float32) * fk,
                           np.asarray(Wv, np.float32) * fv], axis=1)
    wt = np.ascontiguousarray(
        wcat.reshape(4, 128, 1024).transpose(1, 0, 2)).astype(np.float32)

    shq_f = shq * sqr[g256]
    shk_f = shk * sqk[g256] / sqr[g256]
    shv_f = shv * ssv
    shifts = np.stack([shq_f[:128], shq_f[128:], shk_f[:128], shk_f[128:],
                       shv_f[0:128], shv_f[128:256], shv_f[256:384],
                       shv_f[384:512]], axis=1)
    cvd = np.ascontiguousarray(shifts.astype(np.float32))      # [128, 8]

    i = np.arange(K)
    qr_t = np.asarray(q_rel, np.float32)[:, 0, :]              # [111, 32]
    kr_t = np.asarray(k_rel, np.float32)[:, 0, :]
    vr_t = np.asarray(v_rel, np.float32)[:, 0, :]              # [111, 64]
    idx = 55 + i[:, None] - i[None, :]                         # [i, j]
    qed = np.ascontiguousarray(
        qr_t[idx].transpose(2, 0, 1).reshape(32, PIXB)).astype(np.float32)
    ked = np.ascontiguousarray(
        kr_t[idx].transpose(2, 0, 1).reshape(32, PIXB)).astype(np.float32)
    vrd = np.zeros((128, 512), np.float32)
    vrd[:111] = np.tile(vr_t, (1, G)) * ssve[None, :]
    vrd = vrd.astype(bf16)
    sckr_g = (skr * sqr / sqk).astype(np.float32)              # per g
    scd = np.ascontiguousarray(np.broadcast_to(
        np.repeat(sckr_g, K)[None, :], (IQ, 448))).astype(np.float32)
    outc = (shsv + shsve).astype(np.float32)                   # [512]
    return wt, cvd, qed, ked, vrd, scd, outc


def kernel(x, Wq, Wk, Wv, q_rel, k_rel, v_rel,
           p_q, p_k, p_v, p_qk, p_qr, p_kr, p_sv, p_sve):
    import time
    import ml_dtypes
    from concourse.bass_utils import run_bass_kernel_spmd
    bf16 = ml_dtypes.bfloat16

    if "nc" not in _state:
        _state["nc"] = _build()
    nc = _state["nc"]
    wt, cvd, qed, ked, vrd, scd, outc = _prep_host(
        Wq, Wk, Wv, q_rel, k_rel, v_rel,
        p_q, p_k, p_v, p_qk, p_qr, p_kr, p_sv, p_sve)
    idd = np.eye(128, dtype=np.float32).astype(bf16)

    x = np.asarray(x, np.float32)
    in_maps = []
    for c in range(NCORES):
        xs = x[c * NLOC:(c + 1) * NLOC].reshape(NLOC * PIXB, CIN)
        xth = np.ascontiguousarray(
            xs.T.reshape(4, 128, NLOC * PIXB).transpose(1, 0, 2))
        in_maps.append({"xt": xth, "wt": wt, "qed": qed, "ked": ked,
                        "vrd": vrd, "cvd": cvd, "scd": scd, "idd": idd})

    t0 = time.perf_counter()
    res = run_bass_kernel_spmd(nc, in_maps, list(range(NCORES)), trace=False)
    _state["exec_ns"] = int((time.perf_counter() - t0) * 1e9)
    _state["last_run"] = res

    outs = []
    for c in range(NCORES):
        ytc = np.asarray(res.results[c]["yt"]).astype(np.float32)
        outs.append(ytc.reshape(NLOC, K, K, COUT))
    out = np.concatenate(outs, axis=0) + outc[None, None, None, :]
    return np.ascontiguousarray(out.astype(np.float32))


# revision 3
# speedup vs baseline: 1.2354x; 1.2354x over previous
"""AxialAttention kernel for 8 trn2 NeuronCores — full attention on device.

Data-parallel over batch N=16 -> 2 batches per core. The axon tunnel
(~35MB/s) dominates the measured time, so all compute (projections,
relative-embedding logits, softmax over W, sv/sve contractions) runs on
device; x goes up fp32 (softmax of +-870 logits needs fp32 on the q/k
path) and the final output comes back bf16 (6.4MB/core instead of the
baseline's 25.7MB/core fp32 projection). BN affines are folded into
weights/tables on the host:
  qs = BNq(x@Wq)*s_qr[g]        (folded into Wq', shift_q')
  k' = BNk(x@Wk)*s_qk[g]/s_qr[g]
  v' = BNv(x@Wv)*s_sv[ch]
  logit = qs.k' (qk) + qs.qe (qr) + (k'.ke)*sckr[g] (kr), sckr=s_kr*s_qr/s_qk
  sim = softmax_w(logit);  out = sim@v' + sim@(ve*s_sve) + (shift_sv+shift_sve)
The per-g BN shifts of qk/qr/kr cancel in softmax over w (constant along
w). The final +C[ch] shift is added on the host after the gather. The
q/k/logit/softmax path is fp32; sim/v/out are bf16 (error ~0.4%, well
inside the 2e-2 gate).
"""

import numpy as np

K = 56
G = 8
CIN = 512
COUT = 512
EPS = 1e-3
NCORES = 8
NB = 16
NLOC = NB // NCORES          # 2 batches per core
PIXB = K * K                 # 3136 pixels per batch
NT = 7                       # 3136 = 7 * 448
FT = 448
IQ = 14                      # i-quarter size
_state = {}


def _build():
    import concourse.bacc as bacc
    import concourse.mybir as mybir
    import concourse.tile as tile

    bf = mybir.dt.bfloat16
    f32 = mybir.dt.float32
    EXP = mybir.ActivationFunctionType.Exp
    ADD = mybir.AluOpType.add
    MAX = mybir.AluOpType.max
    AX = mybir.AxisListType.X

    nc = bacc.Bacc("TRN2", target_bir_lowering=False, debug=False)
    xt = nc.declare_dram_parameter("xt", [128, 4, NLOC * PIXB], f32,
                                   isOutput=False)
    wqd = nc.declare_dram_parameter("wqd", [128, 4, 512], f32, isOutput=False)
    wvd = nc.declare_dram_parameter("wvd", [128, 4, 512], bf, isOutput=False)
    qrd = nc.declare_dram_parameter("qrd", [32, 111], f32, isOutput=False)
    krd = nc.declare_dram_parameter("krd", [32, 111], f32, isOutput=False)
    vrd = nc.declare_dram_parameter("vrd", [128, 512], bf, isOutput=False)
    cvd = nc.declare_dram_parameter("cvd", [128, 8], f32, isOutput=False)
    scd = nc.declare_dram_parameter("scd", [IQ, 448], f32, isOutput=False)
    idd = nc.declare_dram_parameter("idd", [128, 128], bf, isOutput=False)
    yt = nc.declare_dram_parameter("yt", [NLOC, K, K * 512], bf, isOutput=True)

    with tile.TileContext(nc) as tc:
        with tc.tile_pool(name="const", bufs=1) as cp, \
             tc.tile_pool(name="dram", bufs=1, space="DRAM") as dp:
            vrt = cp.tile([128, 512], bf)          # [r, g*64+c] scaled v_rel
            nc.sync.dma_start(out=vrt[:], in_=vrd[:])
            cvec = cp.tile([128, 8], f32)          # shifts per ch-tile
            nc.sync.dma_start(out=cvec[:], in_=cvd[:])
            sckr = cp.tile([IQ, 448], f32)
            nc.sync.dma_start(out=sckr[:], in_=scd[:])
            qrl = cp.tile([32, 111], f32)   # q_rel reversed, transposed
            nc.sync.dma_start(out=qrl[:], in_=qrd[:])
            krl = cp.tile([32, 111], f32)   # k_rel transposed
            nc.sync.dma_start(out=krl[:], in_=krd[:])

            for b in range(NLOC):
                qtd = dp.tile([32, G, PIXB], f32, tag="qtd")   # [c,g,(i w)]
                ktd = dp.tile([32, G, PIXB], f32, tag="ktd")
                dv2p = dp.tile([PIXB, 512], bf, tag="dv2p")    # [(j w), ch]
                ds0 = dp.tile([K, IQ * 448], bf, tag="ds0")
                ds1 = dp.tile([K, IQ * 448], bf, tag="ds1")
                ds2 = dp.tile([K, IQ * 448], bf, tag="ds2")
                ds3 = dp.tile([K, IQ * 448], bf, tag="ds3")
                dsim = [ds0, ds1, ds2, ds3]

                # ---- phase A: projection, q/k/v to DRAM in new layouts ---
                with tc.tile_pool(name="pa", bufs=1) as pa, \
                     tc.tile_pool(name="xs", bufs=2) as xsp, \
                     tc.tile_pool(name="tmpp", bufs=3) as tp, \
                     tc.tile_pool(name="psA", bufs=4, space="PSUM") as psA, \
                     tc.tile_pool(name="psT", bufs=4, space="PSUM") as psT:
                    w_sb = pa.tile([128, 4, 1024], f32, tag="w")
                    nc.sync.dma_start(out=w_sb[:, :, 0:512], in_=wqd[:])
                    wv_bf = pa.tile([128, 4, 512], bf, tag="wvb")
                    nc.sync.dma_start(out=wv_bf[:], in_=wvd[:])
                    nc.vector.tensor_copy(out=w_sb[:, :, 512:1024],
                                          in_=wv_bf[:])
                    idn = pa.tile([128, 128], bf, tag="idn")
                    nc.sync.dma_start(out=idn[:], in_=idd[:])
                    for n in range(NT):
                        sl = slice(n * FT, (n + 1) * FT)
                        x_sb = xsp.tile([128, 4, FT], f32, tag="x")
                        nc.sync.dma_start(
                            out=x_sb[:],
                            in_=xt[:, :, b * PIXB + n * FT:
                                   b * PIXB + (n + 1) * FT])
                        for m in range(8):
                            ps = psA.tile([128, FT], f32, tag="ps")
                            for kk in range(4):
                                nc.tensor.matmul(
                                    ps[:],
                                    lhsT=w_sb[:, kk, m * 128:(m + 1) * 128],
                                    rhs=x_sb[:, kk, :],
                                    start=(kk == 0), stop=(kk == 3))
                            if m < 4:  # q (m 0,1) / k (m 2,3): fp32 -> DRAM
                                tmq = tp.tile([128, FT], f32, tag="tmq")
                                nc.vector.tensor_scalar_add(
                                    out=tmq[:], in0=ps[:],
                                    scalar1=cvec[:, m:m + 1])
                                dst = qtd if m < 2 else ktd
                                for gg in range(4):
                                    g = (m % 2) * 4 + gg
                                    nc.sync.dma_start(
                                        out=dst[:, g, sl],
                                        in_=tmq[gg * 32:(gg + 1) * 32, :])
                            else:      # v: bf16, transpose -> pix-major DRAM
                                tmv = tp.tile([128, FT], bf, tag="tmv")
                                nc.vector.tensor_scalar_add(
                                    out=tmv[:], in0=ps[:],
                                    scalar1=cvec[:, m:m + 1])
                                for cc in range(4):
                                    fs = slice(cc * 112, (cc + 1) * 112)
                                    pst = psT.tile([112, 128], bf, tag="pst")
                                    nc.tensor.transpose(pst[:], tmv[:, fs],
                                                        idn[:])
                                    tt2 = tp.tile([112, 128], bf, tag="tt2")
                                    nc.vector.tensor_copy(out=tt2[:],
                                                          in_=pst[:])
                                    nc.sync.dma_start(
                                        out=dv2p[n * FT + cc * 112:
                                                 n * FT + (cc + 1) * 112,
                                                 (m - 4) * 128:(m - 3) * 128],
                                        in_=tt2[:])

                # ---- phase B: logits + softmax per i-quarter -------------
                with tc.tile_pool(name="pbk", bufs=1) as pbk, \
                     tc.tile_pool(name="pbq", bufs=1) as pbq, \
                     tc.tile_pool(name="tmpb", bufs=3) as tb, \
                     tc.tile_pool(name="psqk", bufs=4, space="PSUM") as psqk, \
                     tc.tile_pool(name="psqr", bufs=2, space="PSUM") as psqr, \
                     tc.tile_pool(name="pskr", bufs=2, space="PSUM") as pskr:
                    kt_sb = pbk.tile([32, G, K, K], f32, tag="kt")  # [c,g,j,w]
                    nc.sync.dma_start(
                        out=kt_sb[:],
                        in_=ktd.rearrange("c g (j w) -> c g j w", w=K))
                    for iq in range(4):
                        i0 = iq * IQ
                        psl = slice(i0 * K, (i0 + IQ) * K)
                        qt_q = pbq.tile([32, G, IQ, K], f32, tag="qtq")
                        nc.sync.dma_start(
                            out=qt_q[:],
                            in_=qtd[:, :, psl].rearrange(
                                "c g (i w) -> c g i w", w=K))
                        qe_q = pbq.tile([32, IQ, K], f32, tag="qeq")  # [c,i,j]
                        ke_q = pbq.tile([32, K, IQ], f32, tag="keq")  # [c,j,i]
                        for r in range(IQ):
                            o = 55 - i0 - r
                            nc.vector.tensor_copy(out=qe_q[:, r, :],
                                                  in_=qrl[:, o:o + K])
                            nc.vector.tensor_copy(out=ke_q[:, :, r],
                                                  in_=krl[:, o:o + K])
                        L = pbq.tile([K, IQ, G, K], f32, tag="L")
                        # kr first: initializes all of L via DMA scatter
                        for j in range(K):
                            ps = pskr.tile([IQ, 448], f32, tag="kr")
                            nc.tensor.matmul(
                                ps[:], lhsT=ke_q[:, j, :],
                                rhs=kt_sb[:, :, j, :], start=True, stop=True)
                            krt = tb.tile([IQ, 448], f32, tag="krt")
                            nc.vector.tensor_mul(out=krt[:], in0=ps[:],
                                                 in1=sckr[:])
                            nc.sync.dma_start(out=L[j:j + 1], in_=krt[:])
                        # qr: contiguous adds
                        for r in range(IQ):
                            ps = psqr.tile([K, 448], f32, tag="qr")
                            nc.tensor.matmul(
                                ps[:], lhsT=qe_q[:, r, :],
                                rhs=qt_q[:, :, r, :], start=True, stop=True)
                            Lv = L[:, r].rearrange("p g w -> p (g w)")
                            nc.vector.tensor_add(out=Lv, in0=Lv, in1=ps[:])
                        # qk: strided adds
                        for g in range(G):
                            for w in range(K):
                                ps = psqk.tile([K, IQ], f32, tag="qk")
                                nc.tensor.matmul(
                                    ps[:], lhsT=kt_sb[:, g, :, w],
                                    rhs=qt_q[:, g, :, w],
                                    start=True, stop=True)
                                nc.vector.tensor_add(
                                    out=L[:, :, g, w], in0=L[:, :, g, w],
                                    in1=ps[:])
                        # softmax over w (innermost dim)
                        red = tb.tile([K, IQ, G], f32, tag="red")
                        nc.vector.tensor_reduce(out=red[:], in_=L[:],
                                                axis=AX, op=MAX)
                        nc.vector.tensor_sub(
                            out=L[:], in0=L[:],
                            in1=red[:, :, :, None].broadcast_to(
                                [K, IQ, G, K]))
                        nc.scalar.activation(out=L[:], in_=L[:], func=EXP)
                        nc.vector.tensor_reduce(out=red[:], in_=L[:],
                                                axis=AX, op=ADD)
                        nc.vector.reciprocal(out=red[:], in_=red[:])
                        sim = pbq.tile([K, IQ, G, K], bf, tag="sim")
                        nc.vector.tensor_mul(
                            out=sim[:], in0=L[:],
                            in1=red[:, :, :, None].broadcast_to(
                                [K, IQ, G, K]))
                        nc.sync.dma_start(out=dsim[iq][:], in_=sim[:])

                # ---- phase C: sv + sve + output --------------------------
                with tc.tile_pool(name="pc", bufs=1) as pc, \
                     tc.tile_pool(name="tmpc", bufs=3) as tcp_, \
                     tc.tile_pool(name="psv", bufs=4, space="PSUM") as psv, \
                     tc.tile_pool(name="pse", bufs=4, space="PSUM") as pse:
                    vt_sb = pc.tile([K, K, 512], bf, tag="vt2")
                    nc.sync.dma_start(
                        out=vt_sb[:],
                        in_=dv2p.rearrange("(j w) c -> j w c", j=K))
                    simc = pc.tile([K, 4, IQ, G, K], bf, tag="simc")
                    for iq in range(4):
                        nc.sync.dma_start(out=simc[:, iq], in_=dsim[iq][:])
                    out_sb = pc.tile([64, K, 512], bf, tag="out")
                    # sve per i: 8 matmuls -> [w, (g c)] -> row pad(i) of OUT
                    for i in range(K):
                        iq, r = divmod(i, IQ)
                        vew = tcp_.tile([K, 512], bf, tag="vew")
                        nc.sync.dma_start(out=vew[:],
                                          in_=vrt[55 - i:111 - i, :])
                        ps = pse.tile([K, 512], f32, tag="sve")
                        for g in range(G):
                            nc.tensor.matmul(
                                ps[:, g * 64:(g + 1) * 64],
                                lhsT=simc[:, iq, r, g, :],
                                rhs=vew[:, g * 64:(g + 1) * 64],
                                start=True, stop=True)
                        svet = tcp_.tile([K, 512], bf, tag="svet")
                        nc.vector.tensor_copy(out=svet[:], in_=ps[:])
                        rr = (i % 28) + 32 * (i // 28)
                        nc.sync.dma_start(out=out_sb[rr:rr + 1, :, :],
                                          in_=svet[:])
                    # sv per (w, half): quarters paired -> psum rows 0/32
                    for w in range(K):
                        ps = psv.tile([64, 512], f32, tag="sv")
                        for hh in range(2):
                            for g in range(G):
                                nc.tensor.matmul(
                                    ps[32 * hh:32 * hh + 28,
                                       g * 64:(g + 1) * 64],
                                    lhsT=simc[:, 2 * hh:2 * hh + 2, :, g, w
                                              ].rearrange("p a b -> p (a b)"),
                                    rhs=vt_sb[:, w, g * 64:(g + 1) * 64],
                                    start=True, stop=True)
                        nc.vector.tensor_add(out=out_sb[:, w, :],
                                             in0=out_sb[:, w, :], in1=ps[:])
                    nc.sync.dma_start(out=yt[b, 0:28], in_=out_sb[0:28])
                    nc.sync.dma_start(out=yt[b, 28:K], in_=out_sb[32:60])
    nc.compile()
    return nc


def _prep_host(Wq, Wk, Wv, q_rel, k_rel, v_rel,
               p_q, p_k, p_v, p_qk, p_qr, p_kr, p_sv, p_sve):
    import ml_dtypes
    bf16 = ml_dtypes.bfloat16

    def aff(p):
        g, b, m, v = np.asarray(p, np.float32)
        s = g / np.sqrt(v + EPS)
        return s, b - m * s

    sq, shq = aff(p_q); sk, shk = aff(p_k); sv_, shv = aff(p_v)
    sqk, _ = aff(p_qk); sqr, _ = aff(p_qr); skr, _ = aff(p_kr)
    ssv, shsv = aff(p_sv); ssve, shsve = aff(p_sve)
    g256 = np.repeat(np.arange(G), 32)
    g512 = np.repeat(np.arange(G), 64)

    fq = sq * sqr[g256]
    fk = sk * sqk[g256] / sqr[g256]
    fv = sv_ * ssv
    wcat = np.concatenate([np.asarray(Wq, np.float32) * fq,
                           np.asarray(Wk, np.float32) * fk,
                           np.asarray(Wv, np.float32) * fv], axis=1)
    wtr = wcat.reshape(4, 128, 1024).transpose(1, 0, 2)
    wqd = np.ascontiguousarray(wtr[:, :, 0:512]).astype(np.float32)
    wvd = np.ascontiguousarray(wtr[:, :, 512:1024]).astype(bf16)

    shq_f = shq * sqr[g256]
    shk_f = shk * sqk[g256] / sqr[g256]
    shv_f = shv * ssv
    shifts = np.stack([shq_f[:128], shq_f[128:], shk_f[:128], shk_f[128:],
                       shv_f[0:128], shv_f[128:256], shv_f[256:384],
                       shv_f[384:512]], axis=1)
    cvd = np.ascontiguousarray(shifts.astype(np.float32))      # [128, 8]

    i = np.arange(K)
    qr_t = np.asarray(q_rel, np.float32)[:, 0, :]              # [111, 32]
    kr_t = np.asarray(k_rel, np.float32)[:, 0, :]
    vr_t = np.asarray(v_rel, np.float32)[:, 0, :]              # [111, 64]
    qrd = np.ascontiguousarray(qr_t[::-1].T).astype(np.float32)  # [32, 111]
    krd = np.ascontiguousarray(kr_t.T).astype(np.float32)
    vrd = np.zeros((128, 512), np.float32)
    vrd[:111] = np.tile(vr_t, (1, G)) * ssve[None, :]
    vrd = vrd.astype(bf16)
    sckr_g = (skr * sqr / sqk).astype(np.float32)              # per g
    scd = np.ascontiguousarray(np.broadcast_to(
        np.repeat(sckr_g, K)[None, :], (IQ, 448))).astype(np.float32)
    outc = (shsv + shsve).astype(np.float32)                   # [512]
    return wqd, wvd, cvd, qrd, krd, vrd, scd, outc


def kernel(x, Wq, Wk, Wv, q_rel, k_rel, v_rel,
           p_q, p_k, p_v, p_qk, p_qr, p_kr, p_sv, p_sve):
    import time
    import ml_dtypes
    from concourse.bass_utils import run_bass_kernel_spmd
    bf16 = ml_dtypes.bfloat16

    if "nc" not in _state:
        _state["nc"] = _build()
    nc = _state["nc"]
    wqd, wvd, cvd, qrd, krd, vrd, scd, outc = _prep_host(
        Wq, Wk, Wv, q_rel, k_rel, v_rel,
        p_q, p_k, p_v, p_qk, p_qr, p_kr, p_sv, p_sve)
    idd = np.eye(128, dtype=np.float32).astype(bf16)

    x = np.asarray(x, np.float32)
    in_maps = []
    for c in range(NCORES):
        xs = x[c * NLOC:(c + 1) * NLOC].reshape(NLOC * PIXB, CIN)
        xth = np.ascontiguousarray(
            xs.T.reshape(4, 128, NLOC * PIXB).transpose(1, 0, 2))
        in_maps.append({"xt": xth, "wqd": wqd, "wvd": wvd, "qrd": qrd,
                        "krd": krd, "vrd": vrd, "cvd": cvd, "scd": scd,
                        "idd": idd})

    t0 = time.perf_counter()
    res = run_bass_kernel_spmd(nc, in_maps, list(range(NCORES)), trace=False)
    _state["exec_ns"] = int((time.perf_counter() - t0) * 1e9)
    _state["last_run"] = res

    outs = []
    for c in range(NCORES):
        ytc = np.asarray(res.results[c]["yt"]).astype(np.float32)
        outs.append(ytc.reshape(NLOC, K, K, COUT))
    out = np.concatenate(outs, axis=0) + outc[None, None, None, :]
    return np.ascontiguousarray(out.astype(np.float32))
